# revision 6
# baseline (speedup 1.0000x reference)
"""LoRA attention kernel for 8 Trainium2 NeuronCores.

Sharding: data-parallel over batch B=2 (cores 0-3 -> b=0, cores 4-7 -> b=1),
tensor-parallel over heads within each batch group (4 heads/core). The LoRA
low-rank paths and q/v base linears are folded host-side into one effective
qkv projection weight. Attention is computed with scores transposed
(ST[m, n]) so that the softmax denominator and key-padding mask both fold
into the P@V matmul via an augmented v column, and the P@V contraction runs
without transposing the (huge) probability matrix. The per-head attention
outputs (still transposed, [d, n]) are AllGathered across the 4-core group,
and each core computes a 256-row slice of the output projection.
"""

import sys
from contextlib import ExitStack

import numpy as np

for _p in ("/opt/trn_rl_repo", "/opt/trn_rl_repo/concourse"):
    if _p not in sys.path:
        sys.path.insert(0, _p)

import concourse.bass as bass
import concourse.mybir as mybir
import concourse.tile as tile
from concourse import bacc
from concourse import bass_utils
from concourse.masks import make_identity

F32 = mybir.dt.float32
F32R = mybir.dt.float32r
EXP = mybir.ActivationFunctionType.Exp

H, D, DIM, R = 16, 64, 1024, 10
B, N = 2, 2048
NCORES = 8
GROUPS = [[0, 1, 2, 3], [4, 5, 6, 7]]
HPC = H // 4          # heads per core
HD = HPC * D          # 256 qkv rows per core per projection
ATT = float(D) ** -0.5
LS = 1.0 / R

KT = DIM // 128       # 8 contraction tiles
NT = N // 128         # 16 token tiles
NCH = N // 512        # 4 moving chunks of 512
IT = (3 * HD) // 128  # 6 projection row tiles

# test harness hooks
TRACE = False
TRACE_DIR = None
LAST_RESULTS = None

_NC_CACHE = None


def _build_nc():
    nc = bacc.Bacc(None, target_bir_lowering=False, num_devices=NCORES)

    xT = nc.dram_tensor("xT", (DIM, N), F32R, kind="ExternalInput")
    wT = nc.dram_tensor("wT", (DIM, 3 * HD), F32R, kind="ExternalInput")
    pb = nc.dram_tensor("pb", (3 * HD,), F32, kind="ExternalInput")
    mk = nc.dram_tensor("mk", (N,), F32, kind="ExternalInput")
    woT = nc.dram_tensor("woT", (DIM, HD), F32R, kind="ExternalInput")
    bo = nc.dram_tensor("bo", (HD,), F32, kind="ExternalInput")
    outT = nc.dram_tensor("outT", (HD, N), F32, kind="ExternalOutput")

    agin = nc.dram_tensor("agin", (HD, N), F32R)
    agout = nc.dram_tensor("agout", (DIM, N), F32R)
    recd = nc.dram_tensor("recd", (HPC, N), F32)

    with ExitStack() as ctx:
        tc = ctx.enter_context(tile.TileContext(nc))
        const = ctx.enter_context(tc.tile_pool(name="const", bufs=1))

        ident_f32 = const.tile([128, 128], F32)
        make_identity(nc, ident_f32)
        ident = const.tile([128, 128], F32R)
        nc.vector.tensor_copy(ident, ident_f32)

        pb_sb = const.tile([128, IT], F32)
        nc.sync.dma_start(out=pb_sb, in_=pb[:].rearrange("(i p) -> p i", p=128))
        mk_sb = const.tile([128, NT], F32)
        nc.sync.dma_start(out=mk_sb, in_=mk[:].rearrange("(t p) -> p t", p=128))
        bo_sb = const.tile([128, HD // 128], F32)
        nc.sync.dma_start(out=bo_sb, in_=bo[:].rearrange("(c p) -> p c", p=128))
        woT_sb = const.tile([128, KT, HD], F32R)
        woT_r = woT[:, :].rearrange("(k p) c -> p k c", p=128)
        for k in range(KT):
            nc.sync.dma_start(out=woT_sb[:, k, :], in_=woT_r[:, k, :])

        qkvT = const.tile([128, IT, N], F32R)          # [q0..q255 | k | v] x n
        vsb = const.tile([128, NT, HPC, D + 1], F32R)  # v untransposed + mask col

        # ---- phase 1: fused qkv projection + v transpose/mask ----
        with tc.tile_pool(name="xw", bufs=1) as xw, \
             tc.tile_pool(name="pp_proj", bufs=4, space="PSUM") as ppp, \
             tc.tile_pool(name="pp_vt", bufs=2, space="PSUM") as ppvt:
            xT_sb = xw.tile([128, KT, N], F32R)
            wT_sb = xw.tile([128, KT, 3 * HD], F32R)
            wT_r = wT[:, :].rearrange("(k p) m -> p k m", p=128)
            xT_r = xT[:, :].rearrange("(k p) n -> p k n", p=128)
            for k in range(KT):
                nc.sync.dma_start(out=wT_sb[:, k, :], in_=wT_r[:, k, :])
            for k in range(KT):
                for half in range(2):
                    sl = slice(half * (N // 2), (half + 1) * (N // 2))
                    nc.sync.dma_start(out=xT_sb[:, k, sl], in_=xT_r[:, k, sl])

            for i in range(IT):
                pss = [ppp.tile([128, 512], F32, tag="ps", name=f"ps{i}_{_n}") for _n in range(NCH)]
                for k in range(KT):
                    lhs = wT_sb[:, k, i * 128:(i + 1) * 128]
                    for nch in range(NCH):
                        nc.tensor.matmul(
                            pss[nch],
                            lhsT=lhs,
                            rhs=xT_sb[:, k, nch * 512:(nch + 1) * 512],
                            start=(k == 0),
                            stop=(k == KT - 1),
                        )
                for nch in range(NCH):
                    nc.vector.tensor_scalar_add(
                        qkvT[:, i, nch * 512:(nch + 1) * 512],
                        pss[nch],
                        pb_sb[:, i:i + 1],
                    )

            # transpose vT -> v[m, d], zero masked rows, mask into aug column
            for t in range(NT):
                for j in range(2):
                    vt = ppvt.tile([128, 128], F32R, tag="vt", name=f"vt{t}_{j}")
                    nc.tensor.transpose(
                        vt, qkvT[:, 4 + j, t * 128:(t + 1) * 128], ident
                    )
                    for hh in range(2):
                        h = j * 2 + hh
                        nc.vector.tensor_scalar_mul(
                            vsb[:, t, h, 0:D],
                            vt[:, hh * 64:hh * 64 + 64],
                            mk_sb[:, t:t + 1],
                        )
                for h in range(HPC):
                    nc.vector.tensor_copy(vsb[:, t, h, D:D + 1], mk_sb[:, t:t + 1])

        # ---- phase 2: attention per head ----
        with tc.tile_pool(name="expool", bufs=6) as expool, \
             tc.tile_pool(name="attp", bufs=2) as attp, \
             tc.tile_pool(name="recbp", bufs=2) as recbp, \
             tc.tile_pool(name="recp", bufs=2) as recp, \
             tc.tile_pool(name="pp_o", bufs=1, space="PSUM") as ppo, \
             tc.tile_pool(name="pp_st", bufs=4, space="PSUM") as ppst:
            for h in range(HPC):
                ih, off = h // 2, (h % 2) * 64
                qTh = qkvT[off:off + 64, ih, :]
                kTh = qkvT[off:off + 64, 2 + ih, :]
                op = ppo.tile([128, N], F32, tag="op")
                for t in range(NT):
                    sts = []
                    lhs = kTh[:, t * 128:(t + 1) * 128]
                    for nch in range(NCH):
                        st = ppst.tile([128, 512], F32, tag="st", name=f"st{h}_{t}_{nch}")
                        nc.tensor.matmul(
                            st,
                            lhsT=lhs,
                            rhs=qTh[:, nch * 512:(nch + 1) * 512],
                            start=True,
                            stop=True,
                        )
                        sts.append(st)
                    exs = []
                    for nch in range(NCH):
                        ex = expool.tile([128, 512], F32R, tag="ex", name=f"ex{h}_{t}_{nch}")
                        nc.scalar.activation(ex, sts[nch], EXP)
                        exs.append(ex)
                    vlhs = vsb[:, t, h, :]
                    for nch in range(NCH):
                        nc.tensor.matmul(
                            op[0:D + 1, nch * 512:(nch + 1) * 512],
                            lhsT=vlhs,
                            rhs=exs[nch],
                            start=(t == 0),
                            stop=(t == NT - 1),
                        )
                # normalize rows 0..63 by reciprocal of denom row 64
                rec = recp.tile([1, N], F32, tag="rec")
                nc.vector.reciprocal(rec, op[D:D + 1, :])
                nc.sync.dma_start(out=recd[h:h + 1, :], in_=rec)
                recb = recbp.tile([64, N], F32, tag="recb")
                rsrc = recd[h:h + 1, :]
                nc.sync.dma_start(
                    out=recb,
                    in_=bass.AP(tensor=rsrc.tensor, offset=rsrc.offset,
                                ap=[[0, 64], [1, N]]),
                )
                att = attp.tile([64, N], F32R, tag="att")
                nc.vector.tensor_mul(att, op[0:D, :], recb)
                nc.sync.dma_start(out=agin[h * 64:(h + 1) * 64, :], in_=att)

        # ---- phase 3: AllGather heads within batch group ----
        nc.gpsimd.collective_compute(
            "AllGather",
            mybir.AluOpType.bypass,
            replica_groups=GROUPS,
            ins=[agin[:, :].opt()],
            outs=[agout[:, :].opt()],
        )

        # ---- phase 4: output projection slice ----
        with tc.tile_pool(name="agp", bufs=1) as agp, \
             tc.tile_pool(name="outp", bufs=2) as outp, \
             tc.tile_pool(name="pp_f", bufs=2, space="PSUM") as ppf:
            agT = agp.tile([128, KT, N], F32R)
            ag_r = agout[:, :].rearrange("(k p) n -> p k n", p=128)
            for k in range(KT):
                nc.sync.dma_start(out=agT[:, k, :], in_=ag_r[:, k, :])
            out_r = outT[:, :].rearrange("(c p) n -> p c n", p=128)
            for c in range(HD // 128):
                fp = ppf.tile([128, N], F32, tag="fp")
                for k in range(KT):
                    lhs = woT_sb[:, k, c * 128:(c + 1) * 128]
                    for nch in range(NCH):
                        nc.tensor.matmul(
                            fp[:, nch * 512:(nch + 1) * 512],
                            lhsT=lhs,
                            rhs=agT[:, k, nch * 512:(nch + 1) * 512],
                            start=(k == 0),
                            stop=(k == KT - 1),
                        )
                ot = outp.tile([128, N], F32, tag="ot")
                nc.vector.tensor_scalar_add(ot, fp, bo_sb[:, c:c + 1])
                nc.sync.dma_start(out=out_r[:, c, :], in_=ot)

    nc.finalize()
    return nc


def _prep_core_inputs(inputs, c):
    b, g = c // 4, c % 4
    rows = slice(g * HD, (g + 1) * HD)
    w_qkv = np.asarray(inputs["w_qkv"], np.float32)
    Wq = (w_qkv[0:H * D][rows]
          + np.asarray(inputs["wq_base"], np.float32)[rows]
          + LS * (np.asarray(inputs["wq_B"], np.float32)[rows]
                  @ np.asarray(inputs["wq_A"], np.float32))) * ATT
    Wk = w_qkv[H * D:2 * H * D][rows]
    Wv = (w_qkv[2 * H * D:3 * H * D][rows]
          + np.asarray(inputs["wv_base"], np.float32)[rows]
          + LS * (np.asarray(inputs["wv_B"], np.float32)[rows]
                  @ np.asarray(inputs["wv_A"], np.float32)))
    wTv = np.ascontiguousarray(np.concatenate([Wq, Wk, Wv], 0).T)
    pbv = np.concatenate([
        np.asarray(inputs["bq_base"], np.float32)[rows] * ATT,
        np.zeros(HD, np.float32),
        np.asarray(inputs["bv_base"], np.float32)[rows],
    ]).astype(np.float32)
    xTv = np.ascontiguousarray(np.asarray(inputs["x"], np.float32)[b].T)
    mkv = np.asarray(inputs["mask"]).astype(np.float32)[b]
    woTv = np.ascontiguousarray(
        np.asarray(inputs["w_out"], np.float32)[rows, :].T)
    bov = np.asarray(inputs["b_out"], np.float32)[rows]
    return {"xT": xTv, "wT": wTv, "pb": pbv, "mk": mkv, "woT": woTv, "bo": bov}


def kernel(**inputs):
    global _NC_CACHE, LAST_RESULTS
    if _NC_CACHE is None:
        _NC_CACHE = _build_nc()
    nc = _NC_CACHE
    in_maps = [_prep_core_inputs(inputs, c) for c in range(NCORES)]
    res = bass_utils.run_bass_kernel_spmd(
        nc, in_maps, core_ids=list(range(NCORES)),
        trace=TRACE, tmpdir=TRACE_DIR,
    )
    LAST_RESULTS = res
    out = np.empty((B, N, DIM), np.float32)
    for c in range(NCORES):
        b, g = c // 4, c % 4
        out[b, :, g * HD:(g + 1) * HD] = res.results[c]["outT"].T
    return out
